# revision 16
# baseline (speedup 1.0000x reference)
"""WaveNet-style gated residual conv layer on 8 Trainium2 NeuronCores.

Sharding: data-parallel over batch (B=8 -> 1 batch element per core).

Channel-major layout: one PSUM column per sequence position holds all
128 gate pre-activations (rows 0:64 = tanh-half y_t, rows 64:128 =
0.5 * sigmoid-half y_s; the sigmoid-half conv/cond weights and bias are
pre-scaled by 0.5 host-side).  Because sigmoid(y) = 0.5 + 0.5*tanh(y/2),
a SINGLE Tanh activation over all 128 partitions produces a = tanh(y_t)
and b = tanh(y_s/2); the gate z = a*sigmoid(y_s) = 0.5*a*(1+b).  zraw =
(b + 1) * a is ONE DVE scalar_tensor_tensor op; the 0.5 is folded into
the output weights (device) and the skip unpack (host).  The BIR
verifier requires equal base partitions for SBUF+SBUF input pairs, so b
is first copied to partitions 0:64 (DVE tensor_copy runs at 4x for
packed fp16, so this is cheap); cross-base *outputs* are legal, which
lets zraw land on either partition half of the pair-packed z tile.

Matmul cost on TRN2 is (output free width) x (cycles/row), independent
of contraction depth, so y is computed in 3 matmuls per 512-col chunk
(vs 5 naive):
  m1: K=128  [tap2 x(t)   ; cond ch 0:64 ]   (tile XC, window +16)
  m2: K= 80  [tap1 x(t-8) ; cond ch 64:80]   (tile XS, window +0)
  m3: K= 64  [tap0 x(t-16)]                  (tile XC, window +0)
XC rows 0:64 = x window (host left-padded 16), rows 64:128 = cond
channels 0:64 loaded 16 columns later so both align at one moving
window.  XS rows 0:64 = 8-column-shifted copy of x made on-chip by the
Pool engine (GPSIMD has no PSUM port but SBUF->SBUF tensor_copy is
fine, and Pool is otherwise idle); rows 64:80 = cond channels 64:80.

The 1x1 out-transform is pair-packed: zraw for two cells lands on
partition halves 0:64/64:128 of a shared z tile and one matmul with
blockdiag(0.5*Wout^T) produces both cells' outputs at once (0.5
passes/position).  All four out-matmuls of a window accumulate into one
[128,2048] PSUM tile flushed by a single Act Identity(+bias_out).

Per-core steady state per 512 positions: PE 1792 rows = 747ns, DMA
~775ns (17.9MB fp16 / 360GB/s -> the memory roofline), Act ~756ns,
DVE ~730ns, Pool ~724ns.  All HBM I/O fp16, fp32 PSUM accumulation.
"""

import numpy as np
from contextlib import ExitStack

import concourse.bass as bass
import concourse.tile as tile
from concourse import bacc, mybir
from concourse.bass_utils import run_bass_kernel_spmd

B, C_IN, T = 8, 64, 32768
R, KS, DIL, C_COND = 64, 3, 8, 80
PAD = (KS - 1) * DIL          # 16
W = 8192                      # window = DMA granularity
NW = T // W                   # 8
CELL = 1024                   # activation/psum cell (2 PSUM banks)
CHUNK = 512                   # matmul free width (1 PSUM bank fp32)
F32 = mybir.dt.float32
F16 = mybir.dt.float16
N_CORES = 8
AF = mybir.ActivationFunctionType
ALU = mybir.AluOpType

_cache = {}


def build_module():
    nc = bacc.Bacc(
        "TRN2", target_bir_lowering=False, debug=False, num_devices=N_CORES
    )

    # xch rows 0:64 = [16 zeros, x]; rows 64:128 = [16 zeros, cond 0:64]
    # so ONE DMA per window loads both x and cond_lo with the relative
    # 16-column shift the m1 matmul window expects baked in host-side.
    xch = nc.dram_tensor("xch", [128, T + PAD], F16, kind="ExternalInput")
    chi = nc.dram_tensor("chi", [16, T], F16, kind="ExternalInput")
    ws = nc.dram_tensor("ws", [128, 3 * 128], F16, kind="ExternalInput")
    wo2 = nc.dram_tensor("wo2", [128, 128], F16, kind="ExternalInput")
    b3 = nc.dram_tensor("b3", [128, 2], F32, kind="ExternalInput")
    sk = nc.dram_tensor("sk", [128, T // 2], F16, kind="ExternalOutput")
    oh = nc.dram_tensor("oh", [128, T // 2], F16, kind="ExternalOutput")

    with tile.TileContext(nc) as tc, ExitStack() as ctx:
        const = ctx.enter_context(tc.tile_pool(name="const", bufs=1))
        xcpool = ctx.enter_context(tc.tile_pool(name="xc", bufs=2))
        xspool = ctx.enter_context(tc.tile_pool(name="xs", bufs=2))
        thpool = ctx.enter_context(tc.tile_pool(name="th", bufs=2))
        bcpool = ctx.enter_context(tc.tile_pool(name="bc", bufs=2))
        zpool = ctx.enter_context(tc.tile_pool(name="z", bufs=2))
        obpool = ctx.enter_context(tc.tile_pool(name="ob", bufs=2))
        ypool = ctx.enter_context(
            tc.tile_pool(name="y", bufs=2, space=bass.MemorySpace.PSUM)
        )
        oppool = ctx.enter_context(
            tc.tile_pool(name="op", bufs=1, space=bass.MemorySpace.PSUM)
        )

        w_sb = const.tile([128, 3 * 128], F16)
        wo_sb = const.tile([128, 128], F16)
        b_sb = const.tile([128, 2], F32)

        # --- prologue: PE p-state warm-up (the cost model reaches full
        # clock only after ~3us of continuous PE execution) on zero
        # matmuls while the first loads land; warm psum reuses the
        # (bufs=1) out-transform pool so no extra PSUM bank is needed ---
        warm = const.tile([128, CHUNK], F16)
        nc.vector.memset(warm[:, 0:256], 0.0)
        nc.vector.memset(warm[:, 256:], 0.0)
        wps = ypool.tile([128, CELL], F32, tag="yt")
        nc.tensor.matmul(wps[:, 0:256], warm[:, 0:128], warm[:, 0:256],
                         start=True, stop=True)
        nc.tensor.matmul(wps[:, 0:448], warm[:, 0:128], warm[:, 0:448],
                         start=True, stop=True)
        nc.tensor.matmul(wps[:, 0:480], warm[:, 0:128], warm[:, 0:480],
                         start=True, stop=True)

        xc_t = [None] * NW
        xs_t = [None] * NW

        def emit_loads(wj, pieces):
            """Load window wj.  pieces = list of (lo, hi) window-local
            column ranges (multiples of 512 except the end)."""
            c0 = wj * W
            xc = xcpool.tile([128, W + PAD], F16)
            xs = xspool.tile([128, W + 8], F16)
            xc_t[wj], xs_t[wj] = xc, xs
            cprev = 0
            for (lo, hi) in pieces:
                xlo, xhi = lo, (hi + PAD if hi == W else hi)
                nc.sync.dma_start(xc[:, xlo:xhi], xch[:, c0 + xlo:c0 + xhi])
                nc.sync.dma_start(xs[64:80, lo:hi],
                                  chi[:, c0 + lo:c0 + hi])
                # 8-shifted x copy for tap1 (Pool, SBUF->SBUF); the copy
                # reads 8 columns ahead in xc, so it lags 8 columns
                # behind this piece's x load unless this is the last one
                cl, chh = cprev, (hi + 8 if hi == W else hi - 8)
                if wj > 0 and len(pieces) == 1:
                    # split so the first half is ready before the window
                    # starts (a full-width copy finishes ~2.5us too late)
                    mid = W // 2
                    nc.gpsimd.tensor_copy(xs[0:64, cl:mid],
                                          xc[0:64, cl + 8:mid + 8])
                    nc.gpsimd.tensor_copy(xs[0:64, mid:chh],
                                          xc[0:64, mid + 8:chh + 8])
                else:
                    nc.gpsimd.tensor_copy(xs[0:64, cl:chh],
                                          xc[0:64, cl + 8:chh + 8])
                cprev = chh

        # out-transform matmuls deferred one cell so the PE never waits
        # on Act/DVE to produce z.  FG = cells per flush group (one
        # [128, FG*CHUNK] PSUM out tile, one Act Identity flush).
        FG = 4
        pending = []

        def drain_pending():
            done = []
            for (ztp, xblkp, opp) in pending:
                for q in (0, CHUNK):
                    zoff = CELL * xblkp + q
                    ooff = CELL * (xblkp % (FG // 2)) + q
                    nc.tensor.matmul(opp[:, ooff:ooff + CHUNK], wo_sb[:, :],
                                     ztp[:, zoff:zoff + CHUNK],
                                     start=True, stop=True)
                done.append((ztp, xblkp, opp))
            pending.clear()
            return done

        # first-window loads in pieces (first cell's data lands fast);
        # weights first
        nc.sync.dma_start(w_sb[:, :], ws[:, :])
        nc.sync.dma_start(b_sb[:, :], b3[:, :])
        emit_loads(0, [(0, CELL + CHUNK), (CELL + CHUNK, W // 2), (W // 2, W)])
        nc.sync.dma_start(wo_sb[:, :], wo2[:, :])

        CPW = W // CELL                    # cells per window
        zt = ob = op = None
        obd = wjd = None

        def flush_group(fg, ob_t, op_t, wj_t):
            off = (FG // 2) * CELL * fg
            nc.scalar.activation(ob_t[:, off:off + (FG // 2) * CELL],
                                 op_t[:, :], AF.Identity, bias=b_sb[:, 1:2])
            if fg == CPW // FG - 1:
                # last group of window wj_t: store out on the Act queue
                # (dep is the flush just emitted on the same engine)
                nc.gpsimd.dma_start(
                    oh[:, wj_t * (W // 2):(wj_t + 1) * (W // 2)], ob_t[:, :])

        for g in range(NW * CPW):          # global cell index
            wj, c = divmod(g, CPW)
            if c == 0:
                if wj + 1 < NW:
                    emit_loads(wj + 1, [(0, W)])
                zt = zpool.tile([128, W // 2], F16)
                ob = obpool.tile([128, W // 2], F16)
            xc, xs = xc_t[wj], xs_t[wj]

            yt = ypool.tile([128, CELL], F32, tag="yt")
            for q in (0, CHUNK):
                base = c * CELL + q
                nc.tensor.matmul(yt[:, q:q + CHUNK], w_sb[:, 0:128],
                                 xc[:, base + PAD:base + PAD + CHUNK],
                                 start=True, stop=False)
                nc.tensor.matmul(yt[:, q:q + CHUNK], w_sb[0:80, 128:256],
                                 xs[0:80, base:base + CHUNK],
                                 start=False, stop=False)
                nc.tensor.matmul(yt[:, q:q + CHUNK], w_sb[0:64, 256:384],
                                 xc[0:64, base:base + CHUNK],
                                 start=False, stop=True)
            drained = drain_pending()
            th = thpool.tile([128, CELL], F16)
            nc.scalar.activation(th[:, :], yt[:, :], AF.Tanh, bias=b_sb[:, 0:1])
            for (ztd, xblkd, opd) in drained:
                if xblkd % (FG // 2) == (FG // 2) - 1:
                    flush_group(xblkd // (FG // 2), obd, opd, wjd)
            bc = bcpool.tile([64, CELL], F16)
            nc.vector.tensor_copy(bc[:, :], th[64:128, :])
            ph, xblk = c % 2, c // 2
            nc.vector.scalar_tensor_tensor(
                zt[64 * ph:64 * ph + 64, CELL * xblk:CELL * xblk + CELL],
                bc[:, :], 1.0, th[0:64, :], ALU.add, ALU.mult,
            )
            if ph == 1:
                if xblk % (FG // 2) == 0:
                    op = oppool.tile([128, FG * CHUNK], F32)
                pending.append((zt, xblk, op))
                obd, wjd = ob, wj
            if c == CPW - 1:
                # skip store on SP (fits alongside the loads; DMA
                # queues run concurrently so SP+Pool give ~2x stream BW)
                nc.sync.dma_start(
                    sk[:, wj * (W // 2):(wj + 1) * (W // 2)], zt[:, :])
        drained = drain_pending()
        for (ztd, xblkd, opd) in drained:
            if xblkd % (FG // 2) == (FG // 2) - 1:
                flush_group(xblkd // (FG // 2), obd, opd, wjd)

    nc.compile()
    return nc


def pack_weights(weight_conv, bias_conv, weight_out, bias_out, weight_cond):
    wc3 = weight_conv.astype(np.float32)           # [128, 64, 3]
    wcd = weight_cond[:, :, 0].astype(np.float32)  # [128, 80]
    scale = np.ones((128, 1), np.float32)
    scale[64:] = 0.5                               # sigmoid half: tanh(y/2)
    S = np.zeros((128, 3 * 128), np.float32)
    # m1: tap2 + cond 0:64
    S[0:64, 0:128] = (wc3[:, :, 2] * scale).T
    S[64:128, 0:128] = (wcd[:, 0:64] * scale).T
    # m2: tap1 + cond 64:80
    S[0:64, 128:256] = (wc3[:, :, 1] * scale).T
    S[64:80, 128:256] = (wcd[:, 64:80] * scale).T
    # m3: tap0
    S[0:64, 256:384] = (wc3[:, :, 0] * scale).T
    wo2 = np.zeros((128, 128), np.float32)
    woT = 0.5 * weight_out[:, :, 0].astype(np.float32).T   # zraw = 2z
    wo2[0:64, 0:64] = woT
    wo2[64:128, 64:128] = woT
    b3 = np.zeros((128, 2), np.float32)
    b3[0:64, 0] = bias_conv[0:64]
    b3[64:128, 0] = 0.5 * bias_conv[64:128]
    b3[0:64, 1] = bias_out
    b3[64:128, 1] = bias_out
    return S.astype(np.float16), wo2.astype(np.float16), b3


def make_in_maps(x, cond, weight_conv, bias_conv, weight_out, bias_out,
                 weight_cond):
    S, wo2, b3 = pack_weights(weight_conv, bias_conv, weight_out, bias_out,
                              weight_cond)
    pad = np.zeros((128, PAD), np.float16)
    in_maps = []
    for b in range(B):
        body = np.concatenate(
            [x[b].astype(np.float16), cond[b, 0:64].astype(np.float16)], axis=0)
        xchb = np.concatenate([pad, body], axis=1)
        in_maps.append({
            "xch": np.ascontiguousarray(xchb),
            "chi": np.ascontiguousarray(cond[b, 64:80].astype(np.float16)),
            "ws": S, "wo2": wo2, "b3": b3,
        })
    return in_maps


def _unpack(a2):
    # [128, T/2] -> [64, T]; partition p = 64*ph + chan; col =
    # CELL*k + n where k is the global cell-pair index;
    # t = 2*CELL*k + CELL*ph + n
    a = a2.astype(np.float32).reshape(2, 64, T // (2 * CELL), CELL)
    return a.transpose(1, 2, 0, 3).reshape(64, T)


def unpack_outputs(results):
    output = np.empty((B, R, T), np.float32)
    skip = np.empty((B, R, T), np.float32)
    for b in range(B):
        output[b] = _unpack(results[b]["oh"])
        skip[b] = _unpack(results[b]["sk"]) * 0.5    # z = 0.5 * zraw
    return output, skip


def kernel(**inputs):
    inputs = {k: np.asarray(v, dtype=np.float32) for k, v in inputs.items()}
    if "nc" not in _cache:
        _cache["nc"] = build_module()
    nc = _cache["nc"]
    in_maps = make_in_maps(**inputs)
    res = run_bass_kernel_spmd(nc, in_maps, list(range(N_CORES)))
    return unpack_outputs(res.results)


# revision 17
# speedup vs baseline: 1.0397x; 1.0397x over previous
"""WaveNet-style gated residual conv layer on 8 Trainium2 NeuronCores.

Sharding: data-parallel over batch (B=8 -> 1 batch element per core).

Channel-major layout: one PSUM column per sequence position holds all
128 gate pre-activations (rows 0:64 = tanh-half y_t, rows 64:128 =
0.5 * sigmoid-half y_s; the sigmoid-half conv/cond weights and bias are
pre-scaled by 0.5 host-side).  Because sigmoid(y) = 0.5 + 0.5*tanh(y/2),
a SINGLE Tanh activation over all 128 partitions produces a = tanh(y_t)
and b = tanh(y_s/2); the gate z = a*sigmoid(y_s) = 0.5*a*(1+b).  zraw =
(b + 1) * a is ONE DVE scalar_tensor_tensor op; the 0.5 is folded into
the output weights (device) and the skip unpack (host).  The BIR
verifier requires equal base partitions for SBUF+SBUF input pairs, so b
is first copied to partitions 0:64 (DVE tensor_copy runs at 4x for
packed fp16, so this is cheap); cross-base *outputs* are legal, which
lets zraw land on either partition half of the pair-packed z tile.

Matmul cost on TRN2 is (output free width) x (cycles/row), independent
of contraction depth, so y is computed in 3 matmuls per 512-col chunk
(vs 5 naive):
  m1: K=128  [tap2 x(t)   ; cond ch 0:64 ]   (tile XC, window +16)
  m2: K= 80  [tap1 x(t-8) ; cond ch 64:80]   (tile XS, window +0)
  m3: K= 64  [tap0 x(t-16)]                  (tile XC, window +0)
XC rows 0:64 = x window (host left-padded 16), rows 64:128 = cond
channels 0:64 loaded 16 columns later so both align at one moving
window.  XS rows 0:64 = 8-column-shifted copy of x made on-chip by the
Pool engine (GPSIMD has no PSUM port but SBUF->SBUF tensor_copy is
fine, and Pool is otherwise idle); rows 64:80 = cond channels 64:80.

The 1x1 out-transform is pair-packed: zraw for two cells lands on
partition halves 0:64/64:128 of a shared z tile and one matmul with
blockdiag(0.5*Wout^T) produces both cells' outputs at once (0.5
passes/position).  All four out-matmuls of a window accumulate into one
[128,2048] PSUM tile flushed by a single Act Identity(+bias_out).

Per-core steady state per 512 positions: PE 1792 rows = 747ns, DMA
~775ns (17.9MB fp16 / 360GB/s -> the memory roofline), Act ~756ns,
DVE ~730ns, Pool ~724ns.  All HBM I/O fp16, fp32 PSUM accumulation.
"""

import numpy as np
from contextlib import ExitStack

import concourse.bass as bass
import concourse.tile as tile
from concourse import bacc, mybir
from concourse.bass_utils import run_bass_kernel_spmd

B, C_IN, T = 8, 64, 32768
R, KS, DIL, C_COND = 64, 3, 8, 80
PAD = (KS - 1) * DIL          # 16
W = 8192                      # window = DMA granularity
NW = T // W                   # 8
CELL = 1024                   # activation/psum cell (2 PSUM banks)
CHUNK = 512                   # matmul free width (1 PSUM bank fp32)
F32 = mybir.dt.float32
F16 = mybir.dt.float16
N_CORES = 8
AF = mybir.ActivationFunctionType
ALU = mybir.AluOpType

_cache = {}


def build_module():
    nc = bacc.Bacc(
        "TRN2", target_bir_lowering=False, debug=False, num_devices=N_CORES
    )

    # xch rows 0:64 = [16 zeros, x]; rows 64:128 = [16 zeros, cond 0:64]
    # so ONE DMA per window loads both x and cond_lo with the relative
    # 16-column shift the m1 matmul window expects baked in host-side.
    xch = nc.dram_tensor("xch", [128, T + PAD], F16, kind="ExternalInput")
    chi = nc.dram_tensor("chi", [16, T], F16, kind="ExternalInput")
    ws = nc.dram_tensor("ws", [128, 3 * 128], F16, kind="ExternalInput")
    wo2 = nc.dram_tensor("wo2", [128, 128], F16, kind="ExternalInput")
    b3 = nc.dram_tensor("b3", [128, 2], F32, kind="ExternalInput")
    sk = nc.dram_tensor("sk", [128, T // 2], F16, kind="ExternalOutput")
    oh = nc.dram_tensor("oh", [128, T // 2], F16, kind="ExternalOutput")

    with tile.TileContext(nc) as tc, ExitStack() as ctx:
        const = ctx.enter_context(tc.tile_pool(name="const", bufs=1))
        xcpool = ctx.enter_context(tc.tile_pool(name="xc", bufs=2))
        xspool = ctx.enter_context(tc.tile_pool(name="xs", bufs=2))
        thpool = ctx.enter_context(tc.tile_pool(name="th", bufs=3))
        bcpool = ctx.enter_context(tc.tile_pool(name="bc", bufs=3))
        zpool = ctx.enter_context(tc.tile_pool(name="z", bufs=2))
        obpool = ctx.enter_context(tc.tile_pool(name="ob", bufs=2))
        ypool = ctx.enter_context(
            tc.tile_pool(name="y", bufs=3, space=bass.MemorySpace.PSUM)
        )
        oppool = ctx.enter_context(
            tc.tile_pool(name="op", bufs=1, space=bass.MemorySpace.PSUM)
        )

        w_sb = const.tile([128, 3 * 128], F16)
        wo_sb = const.tile([128, 128], F16)
        b_sb = const.tile([128, 2], F32)

        # --- prologue: PE p-state warm-up (the cost model reaches full
        # clock only after ~3us of continuous PE execution) on zero
        # matmuls while the first loads land; warm psum reuses the
        # (bufs=1) out-transform pool so no extra PSUM bank is needed ---
        warm = const.tile([128, CHUNK], F16)
        nc.vector.memset(warm[:, 0:256], 0.0)
        nc.vector.memset(warm[:, 256:], 0.0)
        wps = ypool.tile([128, CELL], F32, tag="yt")
        nc.tensor.matmul(wps[:, 0:256], warm[:, 0:128], warm[:, 0:256],
                         start=True, stop=True)
        nc.tensor.matmul(wps[:, 0:448], warm[:, 0:128], warm[:, 0:448],
                         start=True, stop=True)
        nc.tensor.matmul(wps[:, 0:480], warm[:, 0:128], warm[:, 0:480],
                         start=True, stop=True)

        xc_t = [None] * NW
        xs_t = [None] * NW

        def emit_loads(wj, pieces):
            """Load window wj.  pieces = list of (lo, hi) window-local
            column ranges (multiples of 512 except the end)."""
            c0 = wj * W
            xc = xcpool.tile([128, W + PAD], F16)
            xs = xspool.tile([128, W + 8], F16)
            xc_t[wj], xs_t[wj] = xc, xs
            cprev = 0
            for (lo, hi) in pieces:
                xlo, xhi = lo, (hi + PAD if hi == W else hi)
                nc.sync.dma_start(xc[:, xlo:xhi], xch[:, c0 + xlo:c0 + xhi])
                nc.sync.dma_start(xs[64:80, lo:hi],
                                  chi[:, c0 + lo:c0 + hi])
                # 8-shifted x copy for tap1 (Pool, SBUF->SBUF); the copy
                # reads 8 columns ahead in xc, so it lags 8 columns
                # behind this piece's x load unless this is the last one
                cl, chh = cprev, (hi + 8 if hi == W else hi - 8)
                if wj > 0 and len(pieces) == 1:
                    # split so the first half is ready before the window
                    # starts (a full-width copy finishes ~2.5us too late)
                    mid = W // 2
                    nc.gpsimd.tensor_copy(xs[0:64, cl:mid],
                                          xc[0:64, cl + 8:mid + 8])
                    nc.gpsimd.tensor_copy(xs[0:64, mid:chh],
                                          xc[0:64, mid + 8:chh + 8])
                else:
                    nc.gpsimd.tensor_copy(xs[0:64, cl:chh],
                                          xc[0:64, cl + 8:chh + 8])
                cprev = chh

        # out-transform matmuls deferred one cell so the PE never waits
        # on Act/DVE to produce z.  FG = cells per flush group (one
        # [128, FG*CHUNK] PSUM out tile, one Act Identity flush).
        FG = 2
        pending = []

        def drain_pending():
            done = []
            for (ztp, xblkp, opp) in pending:
                for q in (0, CHUNK):
                    zoff = CELL * xblkp + q
                    ooff = CELL * (xblkp % (FG // 2)) + q
                    nc.tensor.matmul(opp[:, ooff:ooff + CHUNK], wo_sb[:, :],
                                     ztp[:, zoff:zoff + CHUNK],
                                     start=True, stop=True)
                done.append((ztp, xblkp, opp))
            pending.clear()
            return done

        # first-window loads in pieces (first cell's data lands fast);
        # weights first
        nc.sync.dma_start(w_sb[:, :], ws[:, :])
        nc.sync.dma_start(b_sb[:, :], b3[:, :])
        emit_loads(0, [(0, CELL + CHUNK), (CELL + CHUNK, W // 2), (W // 2, W)])
        nc.sync.dma_start(wo_sb[:, :], wo2[:, :])

        CPW = W // CELL                    # cells per window
        zt = ob = op = None
        obd = wjd = None

        def flush_group(fg, ob_t, op_t, wj_t):
            off = (FG // 2) * CELL * fg
            nc.scalar.activation(ob_t[:, off:off + (FG // 2) * CELL],
                                 op_t[:, :], AF.Identity, bias=b_sb[:, 1:2])
            if fg == CPW // FG - 1:
                # last group of window wj_t: store out on the Act queue
                # (dep is the flush just emitted on the same engine)
                nc.gpsimd.dma_start(
                    oh[:, wj_t * (W // 2):(wj_t + 1) * (W // 2)], ob_t[:, :])

        for g in range(NW * CPW):          # global cell index
            wj, c = divmod(g, CPW)
            if c == 0:
                if wj + 1 < NW:
                    emit_loads(wj + 1, [(0, W)])
                zt = zpool.tile([128, W // 2], F16)
                ob = obpool.tile([128, W // 2], F16)
            xc, xs = xc_t[wj], xs_t[wj]

            yt = ypool.tile([128, CELL], F32, tag="yt")
            for q in (0, CHUNK):
                base = c * CELL + q
                nc.tensor.matmul(yt[:, q:q + CHUNK], w_sb[:, 0:128],
                                 xc[:, base + PAD:base + PAD + CHUNK],
                                 start=True, stop=False)
                nc.tensor.matmul(yt[:, q:q + CHUNK], w_sb[0:80, 128:256],
                                 xs[0:80, base:base + CHUNK],
                                 start=False, stop=False)
                nc.tensor.matmul(yt[:, q:q + CHUNK], w_sb[0:64, 256:384],
                                 xc[0:64, base:base + CHUNK],
                                 start=False, stop=True)
            drained = drain_pending()
            th = thpool.tile([128, CELL], F16)
            nc.scalar.activation(th[:, :], yt[:, :], AF.Tanh, bias=b_sb[:, 0:1])
            for (ztd, xblkd, opd) in drained:
                if xblkd % (FG // 2) == (FG // 2) - 1:
                    flush_group(xblkd // (FG // 2), obd, opd, wjd)
            bc = bcpool.tile([64, CELL], F16)
            nc.vector.tensor_copy(bc[:, :], th[64:128, :])
            ph, xblk = c % 2, c // 2
            nc.vector.scalar_tensor_tensor(
                zt[64 * ph:64 * ph + 64, CELL * xblk:CELL * xblk + CELL],
                bc[:, :], 1.0, th[0:64, :], ALU.add, ALU.mult,
            )
            if ph == 1:
                if xblk % (FG // 2) == 0:
                    op = oppool.tile([128, FG * CHUNK], F32)
                pending.append((zt, xblk, op))
                obd, wjd = ob, wj
            if c == CPW - 1:
                # skip store on SP (fits alongside the loads; DMA
                # queues run concurrently so SP+Pool give ~2x stream BW)
                nc.sync.dma_start(
                    sk[:, wj * (W // 2):(wj + 1) * (W // 2)], zt[:, :])
        drained = drain_pending()
        for (ztd, xblkd, opd) in drained:
            if xblkd % (FG // 2) == (FG // 2) - 1:
                flush_group(xblkd // (FG // 2), obd, opd, wjd)

    nc.compile()
    return nc


def pack_weights(weight_conv, bias_conv, weight_out, bias_out, weight_cond):
    wc3 = weight_conv.astype(np.float32)           # [128, 64, 3]
    wcd = weight_cond[:, :, 0].astype(np.float32)  # [128, 80]
    scale = np.ones((128, 1), np.float32)
    scale[64:] = 0.5                               # sigmoid half: tanh(y/2)
    S = np.zeros((128, 3 * 128), np.float32)
    # m1: tap2 + cond 0:64
    S[0:64, 0:128] = (wc3[:, :, 2] * scale).T
    S[64:128, 0:128] = (wcd[:, 0:64] * scale).T
    # m2: tap1 + cond 64:80
    S[0:64, 128:256] = (wc3[:, :, 1] * scale).T
    S[64:80, 128:256] = (wcd[:, 64:80] * scale).T
    # m3: tap0
    S[0:64, 256:384] = (wc3[:, :, 0] * scale).T
    wo2 = np.zeros((128, 128), np.float32)
    woT = 0.5 * weight_out[:, :, 0].astype(np.float32).T   # zraw = 2z
    wo2[0:64, 0:64] = woT
    wo2[64:128, 64:128] = woT
    b3 = np.zeros((128, 2), np.float32)
    b3[0:64, 0] = bias_conv[0:64]
    b3[64:128, 0] = 0.5 * bias_conv[64:128]
    b3[0:64, 1] = bias_out
    b3[64:128, 1] = bias_out
    return S.astype(np.float16), wo2.astype(np.float16), b3


def make_in_maps(x, cond, weight_conv, bias_conv, weight_out, bias_out,
                 weight_cond):
    S, wo2, b3 = pack_weights(weight_conv, bias_conv, weight_out, bias_out,
                              weight_cond)
    pad = np.zeros((128, PAD), np.float16)
    in_maps = []
    for b in range(B):
        body = np.concatenate(
            [x[b].astype(np.float16), cond[b, 0:64].astype(np.float16)], axis=0)
        xchb = np.concatenate([pad, body], axis=1)
        in_maps.append({
            "xch": np.ascontiguousarray(xchb),
            "chi": np.ascontiguousarray(cond[b, 64:80].astype(np.float16)),
            "ws": S, "wo2": wo2, "b3": b3,
        })
    return in_maps


def _unpack(a2):
    # [128, T/2] -> [64, T]; partition p = 64*ph + chan; col =
    # CELL*k + n where k is the global cell-pair index;
    # t = 2*CELL*k + CELL*ph + n
    a = a2.astype(np.float32).reshape(2, 64, T // (2 * CELL), CELL)
    return a.transpose(1, 2, 0, 3).reshape(64, T)


def unpack_outputs(results):
    output = np.empty((B, R, T), np.float32)
    skip = np.empty((B, R, T), np.float32)
    for b in range(B):
        output[b] = _unpack(results[b]["oh"])
        skip[b] = _unpack(results[b]["sk"]) * 0.5    # z = 0.5 * zraw
    return output, skip


def kernel(**inputs):
    inputs = {k: np.asarray(v, dtype=np.float32) for k, v in inputs.items()}
    if "nc" not in _cache:
        _cache["nc"] = build_module()
    nc = _cache["nc"]
    in_maps = make_in_maps(**inputs)
    res = run_bass_kernel_spmd(nc, in_maps, list(range(N_CORES)))
    return unpack_outputs(res.results)


# revision 18
# speedup vs baseline: 1.0919x; 1.0502x over previous
"""WaveNet-style gated residual conv layer on 8 Trainium2 NeuronCores.

Sharding: data-parallel over batch (B=8 -> 1 batch element per core).

Channel-major layout: one PSUM column per sequence position holds all
128 gate pre-activations (rows 0:64 = tanh-half y_t, rows 64:128 =
0.5 * sigmoid-half y_s; the sigmoid-half conv/cond weights and bias are
pre-scaled by 0.5 host-side).  Because sigmoid(y) = 0.5 + 0.5*tanh(y/2),
a SINGLE Tanh activation over all 128 partitions produces a = tanh(y_t)
and b = tanh(y_s/2); the gate z = a*sigmoid(y_s) = 0.5*a*(1+b).  zraw =
(b + 1) * a is ONE DVE scalar_tensor_tensor op; the 0.5 is folded into
the output weights (device) and the skip unpack (host).  The BIR
verifier requires equal base partitions for SBUF+SBUF input pairs, so b
is first copied to partitions 0:64 (DVE tensor_copy runs at 4x for
packed fp16, so this is cheap); cross-base *outputs* are legal, which
lets zraw land on either partition half of the pair-packed z tile.

Matmul cost on TRN2 is (output free width) x (cycles/row), independent
of contraction depth, so y is computed in 3 matmuls per 512-col chunk
(vs 5 naive):
  m1: K=128  [tap2 x(t)   ; cond ch 0:64 ]   (tile XC, window +16)
  m2: K= 80  [tap1 x(t-8) ; cond ch 64:80]   (tile XS, window +0)
  m3: K= 64  [tap0 x(t-16)]                  (tile XC, window +0)
XC rows 0:64 = x window (host left-padded 16), rows 64:128 = cond
channels 0:64 loaded 16 columns later so both align at one moving
window.  XS rows 0:64 = 8-column-shifted copy of x made on-chip by the
Pool engine (GPSIMD has no PSUM port but SBUF->SBUF tensor_copy is
fine, and Pool is otherwise idle); rows 64:80 = cond channels 64:80.

The 1x1 out-transform is pair-packed: zraw for two cells lands on
partition halves 0:64/64:128 of a shared z tile and one matmul with
blockdiag(0.5*Wout^T) produces both cells' outputs at once (0.5
passes/position).  All four out-matmuls of a window accumulate into one
[128,2048] PSUM tile flushed by a single Act Identity(+bias_out).

Per-core steady state per 512 positions: PE 1792 rows = 747ns, DMA
~775ns (17.9MB fp16 / 360GB/s -> the memory roofline), Act ~756ns,
DVE ~730ns, Pool ~724ns.  All HBM I/O fp16, fp32 PSUM accumulation.
"""

import numpy as np
from contextlib import ExitStack

import concourse.bass as bass
import concourse.tile as tile
from concourse import bacc, mybir
from concourse.bass_utils import run_bass_kernel_spmd

B, C_IN, T = 8, 64, 32768
R, KS, DIL, C_COND = 64, 3, 8, 80
PAD = (KS - 1) * DIL          # 16
W = 8192                      # window = DMA granularity
NW = T // W                   # 8
CELL = 1024                   # activation/psum cell (2 PSUM banks)
CHUNK = 512                   # matmul free width (1 PSUM bank fp32)
F32 = mybir.dt.float32
F16 = mybir.dt.float16
N_CORES = 8
AF = mybir.ActivationFunctionType
ALU = mybir.AluOpType

_cache = {}


def build_module():
    nc = bacc.Bacc(
        "TRN2", target_bir_lowering=False, debug=False, num_devices=N_CORES
    )

    # xch rows 0:64 = [16 zeros, x]; rows 64:128 = [16 zeros, cond 0:64]
    # so ONE DMA per window loads both x and cond_lo with the relative
    # 16-column shift the m1 matmul window expects baked in host-side.
    xch = nc.dram_tensor("xch", [128, T + PAD], F16, kind="ExternalInput")
    chi = nc.dram_tensor("chi", [16, T], F16, kind="ExternalInput")
    ws = nc.dram_tensor("ws", [128, 3 * 128], F16, kind="ExternalInput")
    wo2 = nc.dram_tensor("wo2", [128, 128], F16, kind="ExternalInput")
    b3 = nc.dram_tensor("b3", [128, 2], F32, kind="ExternalInput")
    sk = nc.dram_tensor("sk", [128, T // 2], F16, kind="ExternalOutput")
    oh = nc.dram_tensor("oh", [128, T // 2], F16, kind="ExternalOutput")

    with tile.TileContext(nc) as tc, ExitStack() as ctx:
        const = ctx.enter_context(tc.tile_pool(name="const", bufs=1))
        xcpool = ctx.enter_context(tc.tile_pool(name="xc", bufs=2))
        xspool = ctx.enter_context(tc.tile_pool(name="xs", bufs=2))
        thpool = ctx.enter_context(tc.tile_pool(name="th", bufs=3))
        bcpool = ctx.enter_context(tc.tile_pool(name="bc", bufs=3))
        zpool = ctx.enter_context(tc.tile_pool(name="z", bufs=3))
        obpool = ctx.enter_context(tc.tile_pool(name="ob", bufs=3))
        ypool = ctx.enter_context(
            tc.tile_pool(name="y", bufs=3, space=bass.MemorySpace.PSUM)
        )
        oppool = ctx.enter_context(
            tc.tile_pool(name="op", bufs=1, space=bass.MemorySpace.PSUM)
        )

        w_sb = const.tile([128, 3 * 128], F16)
        wo_sb = const.tile([128, 128], F16)
        b_sb = const.tile([128, 2], F32)

        # --- prologue: PE p-state warm-up (the cost model reaches full
        # clock only after ~3us of continuous PE execution) on zero
        # matmuls while the first loads land; warm psum reuses the
        # (bufs=1) out-transform pool so no extra PSUM bank is needed ---
        warm = const.tile([128, CHUNK], F16)
        nc.vector.memset(warm[:, 0:256], 0.0)
        nc.vector.memset(warm[:, 256:], 0.0)
        wps = ypool.tile([128, CELL], F32, tag="yt")
        nc.tensor.matmul(wps[:, 0:256], warm[:, 0:128], warm[:, 0:256],
                         start=True, stop=True)
        nc.tensor.matmul(wps[:, 0:448], warm[:, 0:128], warm[:, 0:448],
                         start=True, stop=True)
        nc.tensor.matmul(wps[:, 0:480], warm[:, 0:128], warm[:, 0:480],
                         start=True, stop=True)

        xc_t = [None] * NW
        xs_t = [None] * NW

        def emit_loads(wj, pieces):
            """Load window wj.  pieces = list of (lo, hi) window-local
            column ranges (multiples of 512 except the end)."""
            c0 = wj * W
            xc = xcpool.tile([128, W + PAD], F16)
            xs = xspool.tile([128, W + 8], F16)
            xc_t[wj], xs_t[wj] = xc, xs
            cprev = 0
            for (lo, hi) in pieces:
                xlo, xhi = lo, (hi + PAD if hi == W else hi)
                nc.sync.dma_start(xc[:, xlo:xhi], xch[:, c0 + xlo:c0 + xhi])
                nc.sync.dma_start(xs[64:80, lo:hi],
                                  chi[:, c0 + lo:c0 + hi])
                # 8-shifted x copy for tap1 (Pool, SBUF->SBUF); the copy
                # reads 8 columns ahead in xc, so it lags 8 columns
                # behind this piece's x load unless this is the last one
                cl, chh = cprev, (hi + 8 if hi == W else hi - 8)
                if wj > 0 and len(pieces) == 1:
                    # split so the first half is ready before the window
                    # starts (a full-width copy finishes ~2.5us too late)
                    mid = W // 2
                    nc.gpsimd.tensor_copy(xs[0:64, cl:mid],
                                          xc[0:64, cl + 8:mid + 8])
                    nc.gpsimd.tensor_copy(xs[0:64, mid:chh],
                                          xc[0:64, mid + 8:chh + 8])
                else:
                    nc.gpsimd.tensor_copy(xs[0:64, cl:chh],
                                          xc[0:64, cl + 8:chh + 8])
                cprev = chh

        # out-transform matmuls deferred one cell so the PE never waits
        # on Act/DVE to produce z.  FG = cells per flush group (one
        # [128, FG*CHUNK] PSUM out tile, one Act Identity flush).
        FG = 2
        pending = []

        def drain_pending():
            done = []
            for (ztp, xblkp, opp) in pending:
                for q in (0, CHUNK):
                    zoff = CELL * xblkp + q
                    ooff = CELL * (xblkp % (FG // 2)) + q
                    nc.tensor.matmul(opp[:, ooff:ooff + CHUNK], wo_sb[:, :],
                                     ztp[:, zoff:zoff + CHUNK],
                                     start=True, stop=True)
                done.append((ztp, xblkp, opp))
            pending.clear()
            return done

        # first-window loads in pieces (first cell's data lands fast);
        # weights first
        nc.sync.dma_start(w_sb[:, :], ws[:, :])
        nc.sync.dma_start(b_sb[:, :], b3[:, :])
        emit_loads(0, [(0, CELL + CHUNK), (CELL + CHUNK, W // 2), (W // 2, W)])
        nc.sync.dma_start(wo_sb[:, :], wo2[:, :])

        CPW = W // CELL                    # cells per window
        zt = ob = op = None
        obd = wjd = None

        def flush_group(fg, ob_t, op_t, wj_t):
            off = (FG // 2) * CELL * fg
            nc.scalar.activation(ob_t[:, off:off + (FG // 2) * CELL],
                                 op_t[:, :], AF.Identity, bias=b_sb[:, 1:2])
            if fg == CPW // FG - 1:
                # last group of window wj_t: store out on the Act queue
                # (dep is the flush just emitted on the same engine)
                nc.gpsimd.dma_start(
                    oh[:, wj_t * (W // 2):(wj_t + 1) * (W // 2)], ob_t[:, :])

        for g in range(NW * CPW):          # global cell index
            wj, c = divmod(g, CPW)
            if c == 0:
                if wj + 1 < NW:
                    emit_loads(wj + 1, [(0, W)])
                zt_prev = zt
                zt = zpool.tile([128, W // 2], F16)
                ob = obpool.tile([128, W // 2], F16)
            xc, xs = xc_t[wj], xs_t[wj]

            yt = ypool.tile([128, CELL], F32, tag="yt")
            for q in (0, CHUNK):
                base = c * CELL + q
                nc.tensor.matmul(yt[:, q:q + CHUNK], w_sb[:, 0:128],
                                 xc[:, base + PAD:base + PAD + CHUNK],
                                 start=True, stop=False)
                nc.tensor.matmul(yt[:, q:q + CHUNK], w_sb[0:80, 128:256],
                                 xs[0:80, base:base + CHUNK],
                                 start=False, stop=False)
                nc.tensor.matmul(yt[:, q:q + CHUNK], w_sb[0:64, 256:384],
                                 xc[0:64, base:base + CHUNK],
                                 start=False, stop=True)
            drained = drain_pending()
            th = thpool.tile([128, CELL], F16)
            nc.scalar.activation(th[:, :], yt[:, :], AF.Tanh, bias=b_sb[:, 0:1])
            for (ztd, xblkd, opd) in drained:
                if xblkd % (FG // 2) == (FG // 2) - 1:
                    flush_group(xblkd // (FG // 2), obd, opd, wjd)
            bc = bcpool.tile([64, CELL], F16)
            nc.vector.tensor_copy(bc[:, :], th[64:128, :])
            ph, xblk = c % 2, c // 2
            nc.vector.scalar_tensor_tensor(
                zt[64 * ph:64 * ph + 64, CELL * xblk:CELL * xblk + CELL],
                bc[:, :], 1.0, th[0:64, :], ALU.add, ALU.mult,
            )
            if ph == 1:
                if xblk % (FG // 2) == 0:
                    op = oppool.tile([128, FG * CHUNK], F32)
                pending.append((zt, xblk, op))
                obd, wjd = ob, wj
            if c == 1 and wj > 0:
                # skip store of the PREVIOUS window on SP, emitted after
                # this window's loads so the SP queue runs [xc, chi, sk]
                # and the loads land with slack
                nc.sync.dma_start(
                    sk[:, (wj - 1) * (W // 2):wj * (W // 2)], zt_prev[:, :])
        nc.sync.dma_start(sk[:, (NW - 1) * (W // 2):], zt[:, :])
        drained = drain_pending()
        for (ztd, xblkd, opd) in drained:
            if xblkd % (FG // 2) == (FG // 2) - 1:
                flush_group(xblkd // (FG // 2), obd, opd, wjd)

    nc.compile()
    return nc


def pack_weights(weight_conv, bias_conv, weight_out, bias_out, weight_cond):
    wc3 = weight_conv.astype(np.float32)           # [128, 64, 3]
    wcd = weight_cond[:, :, 0].astype(np.float32)  # [128, 80]
    scale = np.ones((128, 1), np.float32)
    scale[64:] = 0.5                               # sigmoid half: tanh(y/2)
    S = np.zeros((128, 3 * 128), np.float32)
    # m1: tap2 + cond 0:64
    S[0:64, 0:128] = (wc3[:, :, 2] * scale).T
    S[64:128, 0:128] = (wcd[:, 0:64] * scale).T
    # m2: tap1 + cond 64:80
    S[0:64, 128:256] = (wc3[:, :, 1] * scale).T
    S[64:80, 128:256] = (wcd[:, 64:80] * scale).T
    # m3: tap0
    S[0:64, 256:384] = (wc3[:, :, 0] * scale).T
    wo2 = np.zeros((128, 128), np.float32)
    woT = 0.5 * weight_out[:, :, 0].astype(np.float32).T   # zraw = 2z
    wo2[0:64, 0:64] = woT
    wo2[64:128, 64:128] = woT
    b3 = np.zeros((128, 2), np.float32)
    b3[0:64, 0] = bias_conv[0:64]
    b3[64:128, 0] = 0.5 * bias_conv[64:128]
    b3[0:64, 1] = bias_out
    b3[64:128, 1] = bias_out
    return S.astype(np.float16), wo2.astype(np.float16), b3


def make_in_maps(x, cond, weight_conv, bias_conv, weight_out, bias_out,
                 weight_cond):
    S, wo2, b3 = pack_weights(weight_conv, bias_conv, weight_out, bias_out,
                              weight_cond)
    pad = np.zeros((128, PAD), np.float16)
    in_maps = []
    for b in range(B):
        body = np.concatenate(
            [x[b].astype(np.float16), cond[b, 0:64].astype(np.float16)], axis=0)
        xchb = np.concatenate([pad, body], axis=1)
        in_maps.append({
            "xch": np.ascontiguousarray(xchb),
            "chi": np.ascontiguousarray(cond[b, 64:80].astype(np.float16)),
            "ws": S, "wo2": wo2, "b3": b3,
        })
    return in_maps


def _unpack(a2):
    # [128, T/2] -> [64, T]; partition p = 64*ph + chan; col =
    # CELL*k + n where k is the global cell-pair index;
    # t = 2*CELL*k + CELL*ph + n
    a = a2.astype(np.float32).reshape(2, 64, T // (2 * CELL), CELL)
    return a.transpose(1, 2, 0, 3).reshape(64, T)


def unpack_outputs(results):
    output = np.empty((B, R, T), np.float32)
    skip = np.empty((B, R, T), np.float32)
    for b in range(B):
        output[b] = _unpack(results[b]["oh"])
        skip[b] = _unpack(results[b]["sk"]) * 0.5    # z = 0.5 * zraw
    return output, skip


def kernel(**inputs):
    inputs = {k: np.asarray(v, dtype=np.float32) for k, v in inputs.items()}
    if "nc" not in _cache:
        _cache["nc"] = build_module()
    nc = _cache["nc"]
    in_maps = make_in_maps(**inputs)
    res = run_bass_kernel_spmd(nc, in_maps, list(range(N_CORES)))
    return unpack_outputs(res.results)


# revision 21
# speedup vs baseline: 1.1649x; 1.0668x over previous
"""WaveNet-style gated residual conv layer on 8 Trainium2 NeuronCores.

Sharding: data-parallel over batch (B=8 -> 1 batch element per core).

Channel-major layout: one PSUM column per sequence position holds all
128 gate pre-activations (rows 0:64 = tanh-half y_t, rows 64:128 =
0.5 * sigmoid-half y_s; the sigmoid-half conv/cond weights and bias are
pre-scaled by 0.5 host-side).  Because sigmoid(y) = 0.5 + 0.5*tanh(y/2),
a SINGLE Tanh activation over all 128 partitions produces a = tanh(y_t)
and b = tanh(y_s/2); the gate z = a*sigmoid(y_s) = 0.5*a*(1+b).  zraw =
(b + 1) * a is ONE DVE scalar_tensor_tensor op; the 0.5 is folded into
the output weights (device) and the skip unpack (host).  The BIR
verifier requires equal base partitions for SBUF+SBUF input pairs, so b
is first copied to partitions 0:64 (DVE tensor_copy runs at 4x for
packed fp16, so this is cheap); cross-base *outputs* are legal, which
lets zraw land on either partition half of the pair-packed z tile.

Matmul cost on TRN2 is (output free width) x (cycles/row), independent
of contraction depth, so y is computed in 3 matmuls per 512-col chunk
(vs 5 naive):
  m1: K=128  [tap2 x(t)   ; cond ch 0:64 ]   (tile XC, window +16)
  m2: K= 80  [tap1 x(t-8) ; cond ch 64:80]   (tile XS, window +0)
  m3: K= 64  [tap0 x(t-16)]                  (tile XC, window +0)
XC rows 0:64 = x window (host left-padded 16), rows 64:128 = cond
channels 0:64 loaded 16 columns later so both align at one moving
window.  XS rows 0:64 = 8-column-shifted copy of x made on-chip by the
Pool engine (GPSIMD has no PSUM port but SBUF->SBUF tensor_copy is
fine, and Pool is otherwise idle); rows 64:80 = cond channels 64:80.

The 1x1 out-transform is pair-packed: zraw for two cells lands on
partition halves 0:64/64:128 of a shared z tile and one matmul with
blockdiag(0.5*Wout^T) produces both cells' outputs at once (0.5
passes/position).  All four out-matmuls of a window accumulate into one
[128,2048] PSUM tile flushed by a single Act Identity(+bias_out).

Per-core steady state per 512 positions: PE 1792 rows = 747ns, DMA
~775ns (17.9MB fp16 / 360GB/s -> the memory roofline), Act ~756ns,
DVE ~730ns, Pool ~724ns.  All HBM I/O fp16, fp32 PSUM accumulation.
"""

import numpy as np
from contextlib import ExitStack

import concourse.bass as bass
import concourse.tile as tile
from concourse import bacc, mybir
from concourse.bass_utils import run_bass_kernel_spmd

B, C_IN, T = 8, 64, 32768
R, KS, DIL, C_COND = 64, 3, 8, 80
PAD = (KS - 1) * DIL          # 16
W = 8192                      # window = DMA granularity
NW = T // W                   # 8
CELL = 1024                   # activation/psum cell (2 PSUM banks)
CHUNK = 512                   # matmul free width (1 PSUM bank fp32)
F32 = mybir.dt.float32
F16 = mybir.dt.float16
N_CORES = 8
AF = mybir.ActivationFunctionType
ALU = mybir.AluOpType

_cache = {}


def build_module():
    nc = bacc.Bacc(
        "TRN2", target_bir_lowering=False, debug=False, num_devices=N_CORES
    )

    # xch rows 0:64 = [16 zeros, x]; rows 64:128 = [16 zeros, cond 0:64]
    # so ONE DMA per window loads both x and cond_lo with the relative
    # 16-column shift the m1 matmul window expects baked in host-side.
    xch = nc.dram_tensor("xch", [128, T + PAD], F16, kind="ExternalInput")
    chi = nc.dram_tensor("chi", [16, T], F16, kind="ExternalInput")
    ws = nc.dram_tensor("ws", [128, 3 * 128], F16, kind="ExternalInput")
    wo2 = nc.dram_tensor("wo2", [128, 128], F16, kind="ExternalInput")
    b3 = nc.dram_tensor("b3", [128, 2], F32, kind="ExternalInput")
    sk = nc.dram_tensor("sk", [128, T // 2], F16, kind="ExternalOutput")
    oh = nc.dram_tensor("oh", [128, T // 2], F16, kind="ExternalOutput")

    with tile.TileContext(nc) as tc, ExitStack() as ctx:
        const = ctx.enter_context(tc.tile_pool(name="const", bufs=1))
        xcpool = ctx.enter_context(tc.tile_pool(name="xc", bufs=2))
        xspool = ctx.enter_context(tc.tile_pool(name="xs", bufs=2))
        thpool = ctx.enter_context(tc.tile_pool(name="th", bufs=3))
        bcpool = ctx.enter_context(tc.tile_pool(name="bc", bufs=3))
        zpool = ctx.enter_context(tc.tile_pool(name="z", bufs=3))
        obpool = ctx.enter_context(tc.tile_pool(name="ob", bufs=3))
        ypool = ctx.enter_context(
            tc.tile_pool(name="y", bufs=3, space=bass.MemorySpace.PSUM)
        )
        oppool = ctx.enter_context(
            tc.tile_pool(name="op", bufs=1, space=bass.MemorySpace.PSUM)
        )

        w_sb = const.tile([128, 3 * 128], F16)
        wo_sb = const.tile([128, 128], F16)
        b_sb = const.tile([128, 2], F32)

        # --- prologue: PE p-state warm-up (the cost model reaches full
        # clock only after ~3us of continuous PE execution) on zero
        # matmuls while the first loads land; warm psum reuses the
        # (bufs=1) out-transform pool so no extra PSUM bank is needed ---
        warm = const.tile([128, CHUNK], F16)
        nc.vector.memset(warm[:, 0:256], 0.0)
        nc.vector.memset(warm[:, 256:], 0.0)
        wps = ypool.tile([128, CELL], F32, tag="yt")
        nc.tensor.matmul(wps[:, 0:256], warm[:, 0:128], warm[:, 0:256],
                         start=True, stop=True)
        nc.tensor.matmul(wps[:, 0:448], warm[:, 0:128], warm[:, 0:448],
                         start=True, stop=True)
        nc.tensor.matmul(wps[:, 0:480], warm[:, 0:128], warm[:, 0:480],
                         start=True, stop=True)

        xc_t = [None] * NW
        xs_t = [None] * NW

        def emit_loads(wj, pieces):
            """Load window wj.  pieces = list of (lo, hi) window-local
            column ranges (multiples of 512 except the end)."""
            c0 = wj * W
            xc = xcpool.tile([128, W + PAD], F16)
            xs = xspool.tile([128, W + 8], F16)
            xc_t[wj], xs_t[wj] = xc, xs
            cprev = 0
            for (lo, hi) in pieces:
                xlo, xhi = lo, (hi + PAD if hi == W else hi)
                nc.sync.dma_start(xc[:, xlo:xhi], xch[:, c0 + xlo:c0 + xhi])
                nc.sync.dma_start(xs[64:80, lo:hi],
                                  chi[:, c0 + lo:c0 + hi])
                # 8-shifted x copy for tap1 (Pool, SBUF->SBUF); the copy
                # reads 8 columns ahead in xc, so it lags 8 columns
                # behind this piece's x load unless this is the last one
                cl, chh = cprev, (hi + 8 if hi == W else hi - 8)
                if wj > 0 and len(pieces) == 1:
                    # split so the first half is ready before the window
                    # starts (a full-width copy finishes ~2.5us too late)
                    mid = W // 2
                    nc.gpsimd.tensor_copy(xs[0:64, cl:mid],
                                          xc[0:64, cl + 8:mid + 8])
                    nc.gpsimd.tensor_copy(xs[0:64, mid:chh],
                                          xc[0:64, mid + 8:chh + 8])
                else:
                    nc.gpsimd.tensor_copy(xs[0:64, cl:chh],
                                          xc[0:64, cl + 8:chh + 8])
                cprev = chh

        # out-transform matmuls deferred one cell so the PE never waits
        # on Act/DVE to produce z.  FG = cells per flush group (one
        # [128, FG*CHUNK] PSUM out tile, one Act Identity flush).
        FG = 2
        pending = []

        def drain_pending():
            done = []
            for (ztp, xblkp, opp) in pending:
                for q in (0, CHUNK):
                    zoff = CELL * xblkp + q
                    ooff = CELL * (xblkp % (FG // 2)) + q
                    nc.tensor.matmul(opp[:, ooff:ooff + CHUNK], wo_sb[:, :],
                                     ztp[:, zoff:zoff + CHUNK],
                                     start=True, stop=True)
                done.append((ztp, xblkp, opp))
            pending.clear()
            return done

        # first-window loads: small lead pieces on SP (first cell's data
        # lands ~2.5us in), back half via Pool SWDGE so both DMA queues
        # stream concurrently and the window-1 load can start early
        nc.sync.dma_start(w_sb[:, :], ws[:, :])
        xc0 = xcpool.tile([128, W + PAD], F16)
        xs0 = xspool.tile([128, W + 8], F16)
        xc_t[0], xs_t[0] = xc0, xs0
        nc.sync.dma_start(xc0[:, 0:1040], xch[:, 0:1040])
        nc.sync.dma_start(xs0[64:80, 0:W], chi[:, 0:W])
        nc.sync.dma_start(b_sb[:, :], b3[:, :])
        nc.sync.dma_start(xc0[:, 1040:2560], xch[:, 1040:2560])
        nc.sync.dma_start(wo_sb[:, :], wo2[:, :])
        nc.gpsimd.dma_start(xc0[:, 2560:4608], xch[:, 2560:4608])
        nc.gpsimd.dma_start(xc0[:, 4608:W + PAD], xch[:, 4608:W + PAD])
        for (cl, chh) in ((0, 1032), (1032, 2552), (2552, 4600),
                          (4600, W + 8)):
            nc.gpsimd.tensor_copy(xs0[0:64, cl:chh], xc0[0:64, cl + 8:chh + 8])

        CPW = W // CELL                    # cells per window
        zt = ob = op = None
        obd = wjd = None

        def flush_group(fg, ob_t, op_t, wj_t):
            off = (FG // 2) * CELL * fg
            ngrp = CPW // FG
            nc.scalar.activation(ob_t[:, off:off + (FG // 2) * CELL],
                                 op_t[:, :], AF.Identity, bias=b_sb[:, 1:2])
            base = wj_t * (W // 2)
            if wj_t == NW - 1:
                # last window: store out in two halves to shorten the
                # drain chain
                if fg == ngrp // 2 - 1:
                    nc.gpsimd.dma_start(oh[:, base:base + W // 4],
                                        ob_t[:, 0:W // 4])
                elif fg == ngrp - 1:
                    nc.gpsimd.dma_start(oh[:, base + W // 4:base + W // 2],
                                        ob_t[:, W // 4:])
            elif fg == ngrp - 1:
                nc.gpsimd.dma_start(oh[:, base:base + W // 2], ob_t[:, :])

        for g in range(NW * CPW):          # global cell index
            wj, c = divmod(g, CPW)
            if c == 0:
                if wj + 1 < NW:
                    emit_loads(wj + 1, [(0, W)])
                zt_prev = zt
                zt = zpool.tile([128, W // 2], F16)
                ob = obpool.tile([128, W // 2], F16)
            xc, xs = xc_t[wj], xs_t[wj]

            yt = ypool.tile([128, CELL], F32, tag="yt")
            for q in (0, CHUNK):
                base = c * CELL + q
                nc.tensor.matmul(yt[:, q:q + CHUNK], w_sb[:, 0:128],
                                 xc[:, base + PAD:base + PAD + CHUNK],
                                 start=True, stop=False)
                nc.tensor.matmul(yt[:, q:q + CHUNK], w_sb[0:80, 128:256],
                                 xs[0:80, base:base + CHUNK],
                                 start=False, stop=False)
                nc.tensor.matmul(yt[:, q:q + CHUNK], w_sb[0:64, 256:384],
                                 xc[0:64, base:base + CHUNK],
                                 start=False, stop=True)
            drained = drain_pending()
            th = thpool.tile([128, CELL], F16)
            nc.scalar.activation(th[:, :], yt[:, :], AF.Tanh, bias=b_sb[:, 0:1])
            for (ztd, xblkd, opd) in drained:
                if xblkd % (FG // 2) == (FG // 2) - 1:
                    flush_group(xblkd // (FG // 2), obd, opd, wjd)
            bc = bcpool.tile([64, CELL], F16)
            nc.vector.tensor_copy(bc[:, :], th[64:128, :])
            ph, xblk = c % 2, c // 2
            nc.vector.scalar_tensor_tensor(
                zt[64 * ph:64 * ph + 64, CELL * xblk:CELL * xblk + CELL],
                bc[:, :], 1.0, th[0:64, :], ALU.add, ALU.mult,
            )
            if ph == 1:
                if xblk % (FG // 2) == 0:
                    op = oppool.tile([128, FG * CHUNK], F32)
                pending.append((zt, xblk, op))
                obd, wjd = ob, wj
            if c == 1 and wj > 0:
                # skip store of the PREVIOUS window, emitted after this
                # window's loads so the loads land with slack
                nc.gpsimd.dma_start(
                    sk[:, (wj - 1) * (W // 2):wj * (W // 2)], zt_prev[:, :])
            if wj == NW - 1 and c == 3:
                # last window: store the first half early to shorten the
                # drain chain after the final cell
                nc.gpsimd.dma_start(
                    sk[:, wj * (W // 2):wj * (W // 2) + W // 4],
                    zt[:, 0:W // 4])
        nc.gpsimd.dma_start(sk[:, (NW - 1) * (W // 2) + W // 4:],
                            zt[:, W // 4:])
        drained = drain_pending()
        for (ztd, xblkd, opd) in drained:
            if xblkd % (FG // 2) == (FG // 2) - 1:
                flush_group(xblkd // (FG // 2), obd, opd, wjd)

    nc.compile()
    return nc


def pack_weights(weight_conv, bias_conv, weight_out, bias_out, weight_cond):
    wc3 = weight_conv.astype(np.float32)           # [128, 64, 3]
    wcd = weight_cond[:, :, 0].astype(np.float32)  # [128, 80]
    scale = np.ones((128, 1), np.float32)
    scale[64:] = 0.5                               # sigmoid half: tanh(y/2)
    S = np.zeros((128, 3 * 128), np.float32)
    # m1: tap2 + cond 0:64
    S[0:64, 0:128] = (wc3[:, :, 2] * scale).T
    S[64:128, 0:128] = (wcd[:, 0:64] * scale).T
    # m2: tap1 + cond 64:80
    S[0:64, 128:256] = (wc3[:, :, 1] * scale).T
    S[64:80, 128:256] = (wcd[:, 64:80] * scale).T
    # m3: tap0
    S[0:64, 256:384] = (wc3[:, :, 0] * scale).T
    wo2 = np.zeros((128, 128), np.float32)
    woT = 0.5 * weight_out[:, :, 0].astype(np.float32).T   # zraw = 2z
    wo2[0:64, 0:64] = woT
    wo2[64:128, 64:128] = woT
    b3 = np.zeros((128, 2), np.float32)
    b3[0:64, 0] = bias_conv[0:64]
    b3[64:128, 0] = 0.5 * bias_conv[64:128]
    b3[0:64, 1] = bias_out
    b3[64:128, 1] = bias_out
    return S.astype(np.float16), wo2.astype(np.float16), b3


def make_in_maps(x, cond, weight_conv, bias_conv, weight_out, bias_out,
                 weight_cond):
    S, wo2, b3 = pack_weights(weight_conv, bias_conv, weight_out, bias_out,
                              weight_cond)
    pad = np.zeros((128, PAD), np.float16)
    in_maps = []
    for b in range(B):
        body = np.concatenate(
            [x[b].astype(np.float16), cond[b, 0:64].astype(np.float16)], axis=0)
        xchb = np.concatenate([pad, body], axis=1)
        in_maps.append({
            "xch": np.ascontiguousarray(xchb),
            "chi": np.ascontiguousarray(cond[b, 64:80].astype(np.float16)),
            "ws": S, "wo2": wo2, "b3": b3,
        })
    return in_maps


def _unpack(a2):
    # [128, T/2] -> [64, T]; partition p = 64*ph + chan; col =
    # CELL*k + n where k is the global cell-pair index;
    # t = 2*CELL*k + CELL*ph + n
    a = a2.astype(np.float32).reshape(2, 64, T // (2 * CELL), CELL)
    return a.transpose(1, 2, 0, 3).reshape(64, T)


def unpack_outputs(results):
    output = np.empty((B, R, T), np.float32)
    skip = np.empty((B, R, T), np.float32)
    for b in range(B):
        output[b] = _unpack(results[b]["oh"])
        skip[b] = _unpack(results[b]["sk"]) * 0.5    # z = 0.5 * zraw
    return output, skip


def kernel(**inputs):
    inputs = {k: np.asarray(v, dtype=np.float32) for k, v in inputs.items()}
    if "nc" not in _cache:
        _cache["nc"] = build_module()
    nc = _cache["nc"]
    in_maps = make_in_maps(**inputs)
    res = run_bass_kernel_spmd(nc, in_maps, list(range(N_CORES)))
    return unpack_outputs(res.results)


# revision 22
# speedup vs baseline: 1.3770x; 1.1820x over previous
"""WaveNet-style gated residual conv layer on 8 Trainium2 NeuronCores.

Sharding: data-parallel over batch (B=8 -> 1 batch element per core).

Channel-major layout: one PSUM column per sequence position holds all
128 gate pre-activations (rows 0:64 = tanh-half y_t, rows 64:128 =
0.5 * sigmoid-half y_s; the sigmoid-half conv/cond weights and bias are
pre-scaled by 0.5 host-side).  Because sigmoid(y) = 0.5 + 0.5*tanh(y/2),
a SINGLE Tanh activation over all 128 partitions produces a = tanh(y_t)
and b = tanh(y_s/2); the gate z = a*sigmoid(y_s) = 0.5*a*(1+b).  zraw =
(b + 1) * a is ONE DVE scalar_tensor_tensor op; the 0.5 is folded into
the output weights (device) and the skip unpack (host).  The BIR
verifier requires equal base partitions for SBUF+SBUF input pairs, so b
is first copied to partitions 0:64 (DVE tensor_copy runs at 4x for
packed fp16, so this is cheap); cross-base *outputs* are legal, which
lets zraw land on either partition half of the pair-packed z tile.

Matmul cost on TRN2 is (output free width) x (cycles/row), independent
of contraction depth, so y is computed in 3 matmuls per 512-col chunk
(vs 5 naive):
  m1: K=128  [tap2 x(t)   ; cond ch 0:64 ]   (tile XC, window +16)
  m2: K= 80  [tap1 x(t-8) ; cond ch 64:80]   (tile XS, window +0)
  m3: K= 64  [tap0 x(t-16)]                  (tile XC, window +0)
XC rows 0:64 = x window (host left-padded 16), rows 64:128 = cond
channels 0:64 loaded 16 columns later so both align at one moving
window.  XS rows 0:64 = 8-column-shifted copy of x made on-chip by the
Pool engine (GPSIMD has no PSUM port but SBUF->SBUF tensor_copy is
fine, and Pool is otherwise idle); rows 64:80 = cond channels 64:80.

The 1x1 out-transform is pair-packed: zraw for two cells lands on
partition halves 0:64/64:128 of a shared z tile and one matmul with
blockdiag(0.5*Wout^T) produces both cells' outputs at once (0.5
passes/position).  All four out-matmuls of a window accumulate into one
[128,2048] PSUM tile flushed by a single Act Identity(+bias_out).

Per-core steady state per 512 positions: PE 1792 rows = 747ns, DMA
~775ns (17.9MB fp16 / 360GB/s -> the memory roofline), Act ~756ns,
DVE ~730ns, Pool ~724ns.  All HBM I/O fp16, fp32 PSUM accumulation.
"""

import numpy as np
from contextlib import ExitStack

import concourse.bass as bass
import concourse.tile as tile
from concourse import bacc, mybir
from concourse.bass_utils import run_bass_kernel_spmd

B, C_IN, T = 8, 64, 32768
R, KS, DIL, C_COND = 64, 3, 8, 80
PAD = (KS - 1) * DIL          # 16
W = 8192                      # window = DMA granularity
NW = T // W                   # 8
CELL = 1024                   # activation/psum cell (2 PSUM banks)
CHUNK = 512                   # matmul free width (1 PSUM bank fp32)
F32 = mybir.dt.float32
F16 = mybir.dt.float16
N_CORES = 8
AF = mybir.ActivationFunctionType
ALU = mybir.AluOpType

_cache = {}


def build_module():
    nc = bacc.Bacc(
        "TRN2", target_bir_lowering=False, debug=False, num_devices=N_CORES
    )

    # xch rows 0:64 = [16 zeros, x]; rows 64:128 = [16 zeros, cond 0:64]
    # so ONE DMA per window loads both x and cond_lo with the relative
    # 16-column shift the m1 matmul window expects baked in host-side.
    # xsb rows 0:64 = [8 zeros, x] (the tap1 shift baked in), rows 64:80
    # = cond 64:80 -- the DMA cost model charges by free width only, so
    # an 80-partition load costs the same as 16 partitions and replaces
    # both the narrow cond_hi load and the on-chip x-shift copies.
    xch = nc.dram_tensor("xch", [128, T + PAD], F16, kind="ExternalInput")
    xsb = nc.dram_tensor("xsb", [80, T], F16, kind="ExternalInput")
    ws = nc.dram_tensor("ws", [128, 3 * 128], F16, kind="ExternalInput")
    wo2 = nc.dram_tensor("wo2", [128, 128], F16, kind="ExternalInput")
    b3 = nc.dram_tensor("b3", [128, 2], F32, kind="ExternalInput")
    sk = nc.dram_tensor("sk", [128, T // 2], F16, kind="ExternalOutput")
    oh = nc.dram_tensor("oh", [128, T // 2], F16, kind="ExternalOutput")

    with tile.TileContext(nc) as tc, ExitStack() as ctx:
        const = ctx.enter_context(tc.tile_pool(name="const", bufs=1))
        xcpool = ctx.enter_context(tc.tile_pool(name="xc", bufs=2))
        xspool = ctx.enter_context(tc.tile_pool(name="xs", bufs=2))
        thpool = ctx.enter_context(tc.tile_pool(name="th", bufs=3))
        bcpool = ctx.enter_context(tc.tile_pool(name="bc", bufs=3))
        zpool = ctx.enter_context(tc.tile_pool(name="z", bufs=3))
        obpool = ctx.enter_context(tc.tile_pool(name="ob", bufs=3))
        ypool = ctx.enter_context(
            tc.tile_pool(name="y", bufs=3, space=bass.MemorySpace.PSUM)
        )
        oppool = ctx.enter_context(
            tc.tile_pool(name="op", bufs=1, space=bass.MemorySpace.PSUM)
        )

        w_sb = const.tile([128, 3 * 128], F16)
        wo_sb = const.tile([128, 128], F16)
        b_sb = const.tile([128, 2], F32)

        # --- prologue: PE p-state warm-up (the cost model reaches full
        # clock only after ~3us of continuous PE execution) on zero
        # matmuls while the first loads land; warm psum reuses the
        # (bufs=1) out-transform pool so no extra PSUM bank is needed ---
        warm = const.tile([128, CHUNK], F16)
        nc.vector.memset(warm[:, 0:256], 0.0)
        nc.vector.memset(warm[:, 256:], 0.0)
        wps = ypool.tile([128, CELL], F32, tag="yt")
        nc.tensor.matmul(wps[:, 0:256], warm[:, 0:128], warm[:, 0:256],
                         start=True, stop=True)
        nc.tensor.matmul(wps[:, 0:448], warm[:, 0:128], warm[:, 0:448],
                         start=True, stop=True)
        nc.tensor.matmul(wps[:, 0:480], warm[:, 0:128], warm[:, 0:480],
                         start=True, stop=True)

        xc_t = [None] * NW
        xs_t = [None] * NW

        def emit_loads(wj, pieces):
            """Load window wj.  pieces = list of (lo, hi) window-local
            column ranges."""
            c0 = wj * W
            xc = xcpool.tile([128, W + PAD], F16)
            xs = xspool.tile([80, W], F16)
            xc_t[wj], xs_t[wj] = xc, xs
            for (lo, hi) in pieces:
                xlo, xhi = lo, (hi + PAD if hi == W else hi)
                nc.sync.dma_start(xc[:, xlo:xhi], xch[:, c0 + xlo:c0 + xhi])
                nc.gpsimd.dma_start(xs[:, lo:hi], xsb[:, c0 + lo:c0 + hi])

        # out-transform matmuls deferred one cell so the PE never waits
        # on Act/DVE to produce z.  FG = cells per flush group (one
        # [128, FG*CHUNK] PSUM out tile, one Act Identity flush).
        FG = 2
        pending = []

        def drain_pending():
            done = []
            for (ztp, xblkp, opp) in pending:
                for q in (0, CHUNK):
                    zoff = CELL * xblkp + q
                    ooff = CELL * (xblkp % (FG // 2)) + q
                    nc.tensor.matmul(opp[:, ooff:ooff + CHUNK], wo_sb[:, :],
                                     ztp[:, zoff:zoff + CHUNK],
                                     start=True, stop=True)
                done.append((ztp, xblkp, opp))
            pending.clear()
            return done

        # first-window loads: small lead pieces on SP (first cell's data
        # lands ~2.5us in), back half via Pool SWDGE so both DMA queues
        # stream concurrently and the window-1 load can start early
        nc.sync.dma_start(w_sb[:, :], ws[:, :])
        xc0 = xcpool.tile([128, W + PAD], F16)
        xs0 = xspool.tile([80, W], F16)
        xc_t[0], xs_t[0] = xc0, xs0
        nc.sync.dma_start(xc0[:, 0:1040], xch[:, 0:1040])
        nc.gpsimd.dma_start(xs0[:, 0:1024], xsb[:, 0:1024])
        nc.sync.dma_start(b_sb[:, :], b3[:, :])
        nc.sync.dma_start(xc0[:, 1040:2560], xch[:, 1040:2560])
        nc.gpsimd.dma_start(xs0[:, 1024:2560], xsb[:, 1024:2560])
        nc.sync.dma_start(wo_sb[:, :], wo2[:, :])
        nc.sync.dma_start(xc0[:, 2560:4608], xch[:, 2560:4608])
        nc.gpsimd.dma_start(xs0[:, 2560:4608], xsb[:, 2560:4608])
        nc.sync.dma_start(xc0[:, 4608:W + PAD], xch[:, 4608:W + PAD])
        nc.gpsimd.dma_start(xs0[:, 4608:W], xsb[:, 4608:W])

        CPW = W // CELL                    # cells per window
        zt = ob = op = None
        obd = wjd = None

        def flush_group(fg, ob_t, op_t, wj_t):
            off = (FG // 2) * CELL * fg
            ngrp = CPW // FG
            nc.scalar.activation(ob_t[:, off:off + (FG // 2) * CELL],
                                 op_t[:, :], AF.Identity, bias=b_sb[:, 1:2])
            base = wj_t * (W // 2)
            if wj_t == NW - 1:
                # last window: store out in two halves to shorten the
                # drain chain
                if fg == ngrp // 2 - 1:
                    nc.gpsimd.dma_start(oh[:, base:base + W // 4],
                                        ob_t[:, 0:W // 4])
                elif fg == ngrp - 1:
                    nc.gpsimd.dma_start(oh[:, base + W // 4:base + W // 2],
                                        ob_t[:, W // 4:])
            elif fg == ngrp - 1:
                nc.gpsimd.dma_start(oh[:, base:base + W // 2], ob_t[:, :])

        for g in range(NW * CPW):          # global cell index
            wj, c = divmod(g, CPW)
            if c == 0:
                if wj + 1 < NW:
                    emit_loads(wj + 1, [(0, W)])
                zt_prev = zt
                zt = zpool.tile([128, W // 2], F16)
                ob = obpool.tile([128, W // 2], F16)
            xc, xs = xc_t[wj], xs_t[wj]

            yt = ypool.tile([128, CELL], F32, tag="yt")
            for q in (0, CHUNK):
                base = c * CELL + q
                nc.tensor.matmul(yt[:, q:q + CHUNK], w_sb[:, 0:128],
                                 xc[:, base + PAD:base + PAD + CHUNK],
                                 start=True, stop=False)
                nc.tensor.matmul(yt[:, q:q + CHUNK], w_sb[0:80, 128:256],
                                 xs[:, base:base + CHUNK],
                                 start=False, stop=False)
                nc.tensor.matmul(yt[:, q:q + CHUNK], w_sb[0:64, 256:384],
                                 xc[0:64, base:base + CHUNK],
                                 start=False, stop=True)
            drained = drain_pending()
            th = thpool.tile([128, CELL], F16)
            nc.scalar.activation(th[:, :], yt[:, :], AF.Tanh, bias=b_sb[:, 0:1])
            for (ztd, xblkd, opd) in drained:
                if xblkd % (FG // 2) == (FG // 2) - 1:
                    flush_group(xblkd // (FG // 2), obd, opd, wjd)
            bc = bcpool.tile([64, CELL], F16)
            nc.vector.tensor_copy(bc[:, :], th[64:128, :])
            ph, xblk = c % 2, c // 2
            nc.vector.scalar_tensor_tensor(
                zt[64 * ph:64 * ph + 64, CELL * xblk:CELL * xblk + CELL],
                bc[:, :], 1.0, th[0:64, :], ALU.add, ALU.mult,
            )
            if ph == 1:
                if xblk % (FG // 2) == 0:
                    op = oppool.tile([128, FG * CHUNK], F32)
                pending.append((zt, xblk, op))
                obd, wjd = ob, wj
            if c == 1 and wj > 0:
                # skip store of the PREVIOUS window, emitted after this
                # window's loads so the loads land with slack
                nc.gpsimd.dma_start(
                    sk[:, (wj - 1) * (W // 2):wj * (W // 2)], zt_prev[:, :])
            if wj == NW - 1 and c == 3:
                # last window: store the first half early to shorten the
                # drain chain after the final cell
                nc.gpsimd.dma_start(
                    sk[:, wj * (W // 2):wj * (W // 2) + W // 4],
                    zt[:, 0:W // 4])
        nc.gpsimd.dma_start(sk[:, (NW - 1) * (W // 2) + W // 4:],
                            zt[:, W // 4:])
        drained = drain_pending()
        for (ztd, xblkd, opd) in drained:
            if xblkd % (FG // 2) == (FG // 2) - 1:
                flush_group(xblkd // (FG // 2), obd, opd, wjd)

    nc.compile()
    return nc


def pack_weights(weight_conv, bias_conv, weight_out, bias_out, weight_cond):
    wc3 = weight_conv.astype(np.float32)           # [128, 64, 3]
    wcd = weight_cond[:, :, 0].astype(np.float32)  # [128, 80]
    scale = np.ones((128, 1), np.float32)
    scale[64:] = 0.5                               # sigmoid half: tanh(y/2)
    S = np.zeros((128, 3 * 128), np.float32)
    # m1: tap2 + cond 0:64
    S[0:64, 0:128] = (wc3[:, :, 2] * scale).T
    S[64:128, 0:128] = (wcd[:, 0:64] * scale).T
    # m2: tap1 + cond 64:80
    S[0:64, 128:256] = (wc3[:, :, 1] * scale).T
    S[64:80, 128:256] = (wcd[:, 64:80] * scale).T
    # m3: tap0
    S[0:64, 256:384] = (wc3[:, :, 0] * scale).T
    wo2 = np.zeros((128, 128), np.float32)
    woT = 0.5 * weight_out[:, :, 0].astype(np.float32).T   # zraw = 2z
    wo2[0:64, 0:64] = woT
    wo2[64:128, 64:128] = woT
    b3 = np.zeros((128, 2), np.float32)
    b3[0:64, 0] = bias_conv[0:64]
    b3[64:128, 0] = 0.5 * bias_conv[64:128]
    b3[0:64, 1] = bias_out
    b3[64:128, 1] = bias_out
    return S.astype(np.float16), wo2.astype(np.float16), b3


def make_in_maps(x, cond, weight_conv, bias_conv, weight_out, bias_out,
                 weight_cond):
    S, wo2, b3 = pack_weights(weight_conv, bias_conv, weight_out, bias_out,
                              weight_cond)
    pad = np.zeros((128, PAD), np.float16)
    pad8 = np.zeros((64, 8), np.float16)
    in_maps = []
    for b in range(B):
        xb = x[b].astype(np.float16)
        body = np.concatenate([xb, cond[b, 0:64].astype(np.float16)], axis=0)
        xchb = np.concatenate([pad, body], axis=1)
        xsbb = np.concatenate([
            np.concatenate([pad8, xb[:, :T - 8]], axis=1),
            cond[b, 64:80].astype(np.float16),
        ], axis=0)
        in_maps.append({
            "xch": np.ascontiguousarray(xchb),
            "xsb": np.ascontiguousarray(xsbb),
            "ws": S, "wo2": wo2, "b3": b3,
        })
    return in_maps


def _unpack(a2):
    # [128, T/2] -> [64, T]; partition p = 64*ph + chan; col =
    # CELL*k + n where k is the global cell-pair index;
    # t = 2*CELL*k + CELL*ph + n
    a = a2.astype(np.float32).reshape(2, 64, T // (2 * CELL), CELL)
    return a.transpose(1, 2, 0, 3).reshape(64, T)


def unpack_outputs(results):
    output = np.empty((B, R, T), np.float32)
    skip = np.empty((B, R, T), np.float32)
    for b in range(B):
        output[b] = _unpack(results[b]["oh"])
        skip[b] = _unpack(results[b]["sk"]) * 0.5    # z = 0.5 * zraw
    return output, skip


def kernel(**inputs):
    inputs = {k: np.asarray(v, dtype=np.float32) for k, v in inputs.items()}
    if "nc" not in _cache:
        _cache["nc"] = build_module()
    nc = _cache["nc"]
    in_maps = make_in_maps(**inputs)
    res = run_bass_kernel_spmd(nc, in_maps, list(range(N_CORES)))
    return unpack_outputs(res.results)


# revision 28
# speedup vs baseline: 1.3879x; 1.0079x over previous
"""WaveNet-style gated residual conv layer on 8 Trainium2 NeuronCores.

Sharding: data-parallel over batch (B=8 -> 1 batch element per core).

Channel-major layout: one PSUM column per sequence position holds all
128 gate pre-activations (rows 0:64 = tanh-half y_t, rows 64:128 =
0.5 * sigmoid-half y_s; the sigmoid-half conv/cond weights and bias are
pre-scaled by 0.5 host-side).  Because sigmoid(y) = 0.5 + 0.5*tanh(y/2),
a SINGLE Tanh activation over all 128 partitions produces a = tanh(y_t)
and b = tanh(y_s/2); the gate z = a*sigmoid(y_s) = 0.5*a*(1+b).  zraw =
(b + 1) * a is ONE DVE scalar_tensor_tensor op; the 0.5 is folded into
the output weights (device) and the skip unpack (host).  The BIR
verifier requires equal base partitions for SBUF+SBUF input pairs, so b
is first copied to partitions 0:64 (DVE tensor_copy runs at 4x for
packed fp16, so this is cheap); cross-base *outputs* are legal, which
lets zraw land on either partition half of the pair-packed z tile.

Matmul cost on TRN2 is (output free width) x (cycles/row), independent
of contraction depth, so y is computed in 3 matmuls per 512-col chunk
(vs 5 naive):
  m1: K=128  [tap2 x(t)   ; cond ch 0:64 ]   (tile XC, window +16)
  m2: K= 80  [tap1 x(t-8) ; cond ch 64:80]   (tile XS, window +0)
  m3: K= 64  [tap0 x(t-16)]                  (tile XC, window +0)
XC rows 0:64 = x window (host left-padded 16), rows 64:128 = cond
channels 0:64 loaded 16 columns later so both align at one moving
window.  XS rows 0:64 = 8-column-shifted copy of x made on-chip by the
Pool engine (GPSIMD has no PSUM port but SBUF->SBUF tensor_copy is
fine, and Pool is otherwise idle); rows 64:80 = cond channels 64:80.

The 1x1 out-transform is pair-packed: zraw for two cells lands on
partition halves 0:64/64:128 of a shared z tile and one matmul with
blockdiag(0.5*Wout^T) produces both cells' outputs at once (0.5
passes/position).  All four out-matmuls of a window accumulate into one
[128,2048] PSUM tile flushed by a single Act Identity(+bias_out).

Per-core steady state per 512 positions: PE 1792 rows = 747ns, DMA
~775ns (17.9MB fp16 / 360GB/s -> the memory roofline), Act ~756ns,
DVE ~730ns, Pool ~724ns.  All HBM I/O fp16, fp32 PSUM accumulation.
"""

import numpy as np
from contextlib import ExitStack

import concourse.bass as bass
import concourse.tile as tile
from concourse import bacc, mybir
from concourse.bass_utils import run_bass_kernel_spmd

B, C_IN, T = 8, 64, 32768
R, KS, DIL, C_COND = 64, 3, 8, 80
PAD = (KS - 1) * DIL          # 16
W = 8192                      # window = DMA granularity
NW = T // W                   # 8
CELL = 1024                   # activation/psum cell (2 PSUM banks)
CHUNK = 512                   # matmul free width (1 PSUM bank fp32)
F32 = mybir.dt.float32
F16 = mybir.dt.float16
N_CORES = 8
AF = mybir.ActivationFunctionType
ALU = mybir.AluOpType

_cache = {}


def build_module():
    nc = bacc.Bacc(
        "TRN2", target_bir_lowering=False, debug=False, num_devices=N_CORES
    )

    # xch rows 0:64 = [16 zeros, x]; rows 64:128 = [16 zeros, cond 0:64]
    # so ONE DMA per window loads both x and cond_lo with the relative
    # 16-column shift the m1 matmul window expects baked in host-side.
    # xsb rows 0:64 = [8 zeros, x] (the tap1 shift baked in), rows 64:80
    # = cond 64:80 -- the DMA cost model charges by free width only, so
    # an 80-partition load costs the same as 16 partitions and replaces
    # both the narrow cond_hi load and the on-chip x-shift copies.
    xch = nc.dram_tensor("xch", [128, T + PAD], F16, kind="ExternalInput")
    xsb = nc.dram_tensor("xsb", [80, T], F16, kind="ExternalInput")
    ws = nc.dram_tensor("ws", [128, 3 * 128], F16, kind="ExternalInput")
    wo2 = nc.dram_tensor("wo2", [128, 128], F16, kind="ExternalInput")
    b3 = nc.dram_tensor("b3", [128, 2], F32, kind="ExternalInput")
    sk = nc.dram_tensor("sk", [128, T // 2], F16, kind="ExternalOutput")
    oh = nc.dram_tensor("oh", [128, T // 2], F16, kind="ExternalOutput")

    with tile.TileContext(nc) as tc, ExitStack() as ctx:
        const = ctx.enter_context(tc.tile_pool(name="const", bufs=1))
        xcpool = ctx.enter_context(tc.tile_pool(name="xc", bufs=2))
        xspool = ctx.enter_context(tc.tile_pool(name="xs", bufs=2))
        thpool = ctx.enter_context(tc.tile_pool(name="th", bufs=3))
        bcpool = ctx.enter_context(tc.tile_pool(name="bc", bufs=3))
        zpool = ctx.enter_context(tc.tile_pool(name="z", bufs=3))
        obpool = ctx.enter_context(tc.tile_pool(name="ob", bufs=3))
        ypool = ctx.enter_context(
            tc.tile_pool(name="y", bufs=3, space=bass.MemorySpace.PSUM)
        )
        oppool = ctx.enter_context(
            tc.tile_pool(name="op", bufs=1, space=bass.MemorySpace.PSUM)
        )

        w_sb = const.tile([128, 3 * 128], F16)
        wo_sb = const.tile([128, 128], F16)
        b_sb = const.tile([128, 2], F32)

        # --- prologue: PE p-state warm-up (the cost model reaches full
        # clock only after ~3us of continuous PE execution) on zero
        # matmuls while the first loads land; warm psum reuses the
        # (bufs=1) out-transform pool so no extra PSUM bank is needed ---
        warm = const.tile([128, CHUNK], F16)
        nc.vector.memset(warm[:, 0:256], 0.0)
        nc.vector.memset(warm[:, 256:], 0.0)
        # tiny dummy tanh so the 1.3us activation-table load runs during
        # the prologue DMAs instead of gating the first real tanh
        wact = const.tile([128, 8], F16)
        nc.scalar.activation(wact[:, :], warm[:, 0:8], AF.Tanh)
        wps = ypool.tile([128, CELL], F32, tag="yt")
        nc.tensor.matmul(wps[:, 0:256], warm[:, 0:128], warm[:, 0:256],
                         start=True, stop=True)
        nc.tensor.matmul(wps[:, 0:448], warm[:, 0:128], warm[:, 0:448],
                         start=True, stop=True)
        nc.tensor.matmul(wps[:, 0:480], warm[:, 0:128], warm[:, 0:480],
                         start=True, stop=True)

        xc_t = [None] * NW
        xs_t = [None] * NW

        def emit_loads(wj, pieces):
            """Load window wj.  pieces = list of (lo, hi) window-local
            column ranges."""
            c0 = wj * W
            xc = xcpool.tile([128, W + PAD], F16)
            xs = xspool.tile([80, W], F16)
            xc_t[wj], xs_t[wj] = xc, xs
            for (lo, hi) in pieces:
                xlo, xhi = lo, (hi + PAD if hi == W else hi)
                nc.sync.dma_start(xc[:, xlo:xhi], xch[:, c0 + xlo:c0 + xhi])
                nc.gpsimd.dma_start(xs[:, lo:hi], xsb[:, c0 + lo:c0 + hi])

        # out-transform matmuls deferred one cell so the PE never waits
        # on Act/DVE to produce z.  FG = cells per flush group (one
        # [128, FG*CHUNK] PSUM out tile, one Act Identity flush).
        FG = 2
        pending = []

        def drain_pending():
            done = []
            for (ztp, xblkp, opp) in pending:
                for q in (0, CHUNK):
                    zoff = CELL * xblkp + q
                    ooff = CELL * (xblkp % (FG // 2)) + q
                    nc.tensor.matmul(opp[:, ooff:ooff + CHUNK], wo_sb[:, :],
                                     ztp[:, zoff:zoff + CHUNK],
                                     start=True, stop=True)
                done.append((ztp, xblkp, opp))
            pending.clear()
            return done

        # first-window loads: small lead pieces on SP (first cell's data
        # lands ~2.5us in), back half via Pool SWDGE so both DMA queues
        # stream concurrently and the window-1 load can start early
        nc.sync.dma_start(w_sb[:, :], ws[:, :])
        xc0 = xcpool.tile([128, W + PAD], F16)
        xs0 = xspool.tile([80, W], F16)
        xc_t[0], xs_t[0] = xc0, xs0
        nc.sync.dma_start(xc0[:, 0:1040], xch[:, 0:1040])
        nc.gpsimd.dma_start(xs0[:, 0:1024], xsb[:, 0:1024])
        nc.sync.dma_start(b_sb[:, :], b3[:, :])
        nc.sync.dma_start(xc0[:, 1040:2560], xch[:, 1040:2560])
        nc.gpsimd.dma_start(xs0[:, 1024:2560], xsb[:, 1024:2560])
        nc.sync.dma_start(wo_sb[:, :], wo2[:, :])
        nc.sync.dma_start(xc0[:, 2560:4608], xch[:, 2560:4608])
        nc.gpsimd.dma_start(xs0[:, 2560:4608], xsb[:, 2560:4608])
        nc.sync.dma_start(xc0[:, 4608:W + PAD], xch[:, 4608:W + PAD])
        nc.gpsimd.dma_start(xs0[:, 4608:W], xsb[:, 4608:W])

        CPW = W // CELL                    # cells per window
        zt = ob = op = None
        obd = wjd = None

        def flush_group(fg, ob_t, op_t, wj_t):
            off = (FG // 2) * CELL * fg
            ngrp = CPW // FG
            nc.scalar.activation(ob_t[:, off:off + (FG // 2) * CELL],
                                 op_t[:, :], AF.Identity, bias=b_sb[:, 1:2])
            base = wj_t * (W // 2)
            gw = (FG // 2) * CELL
            if wj_t == NW - 1:
                # last window: store each flush group as soon as it is
                # flushed (the tapered final cell stores itself)
                nc.gpsimd.dma_start(
                    oh[:, base + gw * fg:base + gw * (fg + 1)],
                    ob_t[:, gw * fg:gw * (fg + 1)])
            elif fg == ngrp - 1:
                nc.gpsimd.dma_start(oh[:, base:base + W // 2], ob_t[:, :])

        for g in range(NW * CPW):          # global cell index
            wj, c = divmod(g, CPW)
            if c == 0:
                if wj + 1 < NW:
                    emit_loads(wj + 1, [(0, W)])
                zt_prev = zt
                zt = zpool.tile([128, W // 2], F16)
                ob = obpool.tile([128, W // 2], F16)
            xc, xs = xc_t[wj], xs_t[wj]

            yt = ypool.tile([128, CELL], F32, tag="yt")
            for q in (0, CHUNK):
                base = c * CELL + q
                nc.tensor.matmul(yt[:, q:q + CHUNK], w_sb[:, 0:128],
                                 xc[:, base + PAD:base + PAD + CHUNK],
                                 start=True, stop=False)
                nc.tensor.matmul(yt[:, q:q + CHUNK], w_sb[0:80, 128:256],
                                 xs[:, base:base + CHUNK],
                                 start=False, stop=False)
                nc.tensor.matmul(yt[:, q:q + CHUNK], w_sb[0:64, 256:384],
                                 xc[0:64, base:base + CHUNK],
                                 start=False, stop=True)
            drained = drain_pending()
            th = thpool.tile([128, CELL], F16)
            ph, xblk = c % 2, c // 2
            last = (g == NW * CPW - 1)
            if last:
                # final cell: tanh in 512 halves so the drain chain off
                # the second half is short (ordering keeps Act free of
                # head-of-line waits: both tanh halves precede the flush)
                nc.scalar.activation(th[:, 0:CHUNK], yt[:, 0:CHUNK],
                                     AF.Tanh, bias=b_sb[:, 0:1])
                nc.scalar.activation(th[:, CHUNK:], yt[:, CHUNK:],
                                     AF.Tanh, bias=b_sb[:, 0:1])
            else:
                nc.scalar.activation(th[:, :], yt[:, :], AF.Tanh,
                                     bias=b_sb[:, 0:1])
            for (ztd, xblkd, opd) in drained:
                if xblkd % (FG // 2) == (FG // 2) - 1:
                    flush_group(xblkd // (FG // 2), obd, opd, wjd)
            bc = bcpool.tile([64, CELL], F16)
            if last:
                zb = CELL * xblk
                if xblk % (FG // 2) == 0:
                    op = oppool.tile([128, FG * CHUNK], F32)
                for q in (0, CHUNK):
                    nc.vector.tensor_copy(bc[:, q:q + CHUNK],
                                          th[64:128, q:q + CHUNK])
                    nc.vector.scalar_tensor_tensor(
                        zt[64:128, zb + q:zb + q + CHUNK],
                        bc[:, q:q + CHUNK], 1.0, th[0:64, q:q + CHUNK],
                        ALU.add, ALU.mult,
                    )
                    nc.tensor.matmul(op[:, q:q + CHUNK], wo_sb[:, :],
                                     zt[:, zb + q:zb + q + CHUNK],
                                     start=True, stop=True)
                    nc.gpsimd.dma_start(
                        sk[:, wj * (W // 2) + zb + q:
                           wj * (W // 2) + zb + q + CHUNK],
                        zt[:, zb + q:zb + q + CHUNK])
                for q in (0, CHUNK):
                    nc.scalar.activation(ob[:, zb + q:zb + q + CHUNK],
                                         op[:, q:q + CHUNK], AF.Identity,
                                         bias=b_sb[:, 1:2])
                    nc.gpsimd.dma_start(
                        oh[:, wj * (W // 2) + zb + q:
                           wj * (W // 2) + zb + q + CHUNK],
                        ob[:, zb + q:zb + q + CHUNK])
                continue
            nc.vector.tensor_copy(bc[:, :], th[64:128, :])
            nc.vector.scalar_tensor_tensor(
                zt[64 * ph:64 * ph + 64, CELL * xblk:CELL * xblk + CELL],
                bc[:, :], 1.0, th[0:64, :], ALU.add, ALU.mult,
            )
            if ph == 1:
                if xblk % (FG // 2) == 0:
                    op = oppool.tile([128, FG * CHUNK], F32)
                pending.append((zt, xblk, op))
                obd, wjd = ob, wj
            if c == 1 and wj > 0:
                # skip store of the PREVIOUS window, emitted after this
                # window's loads so the loads land with slack
                nc.gpsimd.dma_start(
                    sk[:, (wj - 1) * (W // 2):wj * (W // 2)], zt_prev[:, :])
            if wj == NW - 1 and c == 3:
                # last window: store the first half early to shorten the
                # drain chain after the final cell
                nc.gpsimd.dma_start(
                    sk[:, wj * (W // 2):wj * (W // 2) + W // 4],
                    zt[:, 0:W // 4])
            if wj == NW - 1 and c == 6:
                nc.gpsimd.dma_start(
                    sk[:, wj * (W // 2) + W // 4:
                       wj * (W // 2) + W // 4 + 2 * CHUNK],
                    zt[:, W // 4:W // 4 + 2 * CHUNK])
        # cells 0..5 of the last window were stored piecewise above; the
        # tapered cell 7's stores cover zt cols 3072:4096 (cells 6+7)

    nc.compile()
    return nc


def pack_weights(weight_conv, bias_conv, weight_out, bias_out, weight_cond):
    wc3 = weight_conv.astype(np.float32)           # [128, 64, 3]
    wcd = weight_cond[:, :, 0].astype(np.float32)  # [128, 80]
    scale = np.ones((128, 1), np.float32)
    scale[64:] = 0.5                               # sigmoid half: tanh(y/2)
    S = np.zeros((128, 3 * 128), np.float32)
    # m1: tap2 + cond 0:64
    S[0:64, 0:128] = (wc3[:, :, 2] * scale).T
    S[64:128, 0:128] = (wcd[:, 0:64] * scale).T
    # m2: tap1 + cond 64:80
    S[0:64, 128:256] = (wc3[:, :, 1] * scale).T
    S[64:80, 128:256] = (wcd[:, 64:80] * scale).T
    # m3: tap0
    S[0:64, 256:384] = (wc3[:, :, 0] * scale).T
    wo2 = np.zeros((128, 128), np.float32)
    woT = 0.5 * weight_out[:, :, 0].astype(np.float32).T   # zraw = 2z
    wo2[0:64, 0:64] = woT
    wo2[64:128, 64:128] = woT
    b3 = np.zeros((128, 2), np.float32)
    b3[0:64, 0] = bias_conv[0:64]
    b3[64:128, 0] = 0.5 * bias_conv[64:128]
    b3[0:64, 1] = bias_out
    b3[64:128, 1] = bias_out
    return S.astype(np.float16), wo2.astype(np.float16), b3


def make_in_maps(x, cond, weight_conv, bias_conv, weight_out, bias_out,
                 weight_cond):
    S, wo2, b3 = pack_weights(weight_conv, bias_conv, weight_out, bias_out,
                              weight_cond)
    pad = np.zeros((128, PAD), np.float16)
    pad8 = np.zeros((64, 8), np.float16)
    in_maps = []
    for b in range(B):
        xb = x[b].astype(np.float16)
        body = np.concatenate([xb, cond[b, 0:64].astype(np.float16)], axis=0)
        xchb = np.concatenate([pad, body], axis=1)
        xsbb = np.concatenate([
            np.concatenate([pad8, xb[:, :T - 8]], axis=1),
            cond[b, 64:80].astype(np.float16),
        ], axis=0)
        in_maps.append({
            "xch": np.ascontiguousarray(xchb),
            "xsb": np.ascontiguousarray(xsbb),
            "ws": S, "wo2": wo2, "b3": b3,
        })
    return in_maps


def _unpack(a2):
    # [128, T/2] -> [64, T]; partition p = 64*ph + chan; col =
    # CELL*k + n where k is the global cell-pair index;
    # t = 2*CELL*k + CELL*ph + n
    a = a2.astype(np.float32).reshape(2, 64, T // (2 * CELL), CELL)
    return a.transpose(1, 2, 0, 3).reshape(64, T)


def unpack_outputs(results):
    output = np.empty((B, R, T), np.float32)
    skip = np.empty((B, R, T), np.float32)
    for b in range(B):
        output[b] = _unpack(results[b]["oh"])
        skip[b] = _unpack(results[b]["sk"]) * 0.5    # z = 0.5 * zraw
    return output, skip


def kernel(**inputs):
    inputs = {k: np.asarray(v, dtype=np.float32) for k, v in inputs.items()}
    if "nc" not in _cache:
        _cache["nc"] = build_module()
    nc = _cache["nc"]
    in_maps = make_in_maps(**inputs)
    res = run_bass_kernel_spmd(nc, in_maps, list(range(N_CORES)))
    return unpack_outputs(res.results)
